# revision 6
# baseline (speedup 1.0000x reference)
"""Trainium2 Bass kernel for nn_ConditionalModuleBGR (histogram binning + MLP).

Strategy: data-parallel over 8 NeuronCores, 2 images (6 slabs of [128, 8192])
per core. Per slab, the 64-bin histogram is computed as a bilinear form on the
Tensor engine (radix 8x8):

  - x16 = fp16(x); hi = floor(8*x16) via exact magic rounding; y = x16 - hi/8.
  - 7 "coarse" step planes  A_h = [hi >= h]   (plus constant ones plane h=0)
  - 7 "fine" step planes    B_l = [y >= l/64] (plus ones plane l=0)
  - PE accumulates P[16h+j, 16l+j'] += sum_p A_h(e[p,j]) * B_l(e[p,j'])
    over 512 chained matmuls per slab (16 element-columns per matmul).
    Same-element products live on the j==j' diagonal blocks.
  - Tiny drain: block-diag mask, one matmul folding the coarse cumulative
    diff, mask+reduce folding the fine diff -> 64-bin histogram lands at
    partition 8h+l, directly in the MLP feature layout.
  - MLP (192 -> 128 relu -> 32) on PE + ACT as in the reference.

Plane generation is spread across DVE (4x fp16 mode), ACT (sigmoid-step),
and GPSIMD so it overlaps the PE stream and the HBM DMA.
"""

import numpy as np

import concourse.bacc as bacc
import concourse.mybir as mybir
import concourse.tile as tile
from concourse import bass_utils

N_CORES = 8
N_IMG = 16
IMG_PER_CORE = N_IMG // N_CORES  # 2
CH = 3
SLABS = IMG_PER_CORE * CH  # 6
P = 128
F = 8192  # 1024*1024 / 128
BINS = 64
FEAT = CH * BINS  # 192
HID = 128
OUT = 32
W1_N = FEAT * HID
B1_N = HID
W2_N = HID * OUT
B2_N = OUT
G_OFF = W1_N + B1_N + W2_N + B2_N
N_PARAMS = 28864

C = 1024            # columns per chunk
NCHK = F // C       # 8 chunks per slab
KPC = C // 16       # 64 matmuls per chunk

F32 = mybir.dt.float32
F16 = mybir.dt.float16
BF16 = mybir.dt.bfloat16
ALU = mybir.AluOpType
ACTF = mybir.ActivationFunctionType

# floor magic: u = 8x - 0.5 + 2^-10 (exact); hi = rne(u) via +/- 3*2^22
S1_ADD = -(1.0 / 16.0 - 2.0 ** -13)
MAGIC = float(3 * 2 ** 22)
ACT_BIAS_U = -0.5 + 2.0 ** -10  # u = 8*x + bias on ACT


def _consts_np() -> np.ndarray:
    """[128, 320] fp32: BD | W4 | M2 packed along free dim."""
    BD = np.zeros((128, 128), np.float32)
    for h in range(8):
        for l in range(8):
            for j in range(16):
                BD[16 * h + j, 16 * l + j] = 1.0
    W4 = np.zeros((128, 64), np.float32)
    for h in range(8):
        for j in range(16):
            for hp in range(8):
                for lp in range(8):
                    v = (1.0 if h == hp else 0.0) - (1.0 if h == hp + 1 else 0.0)
                    if v:
                        W4[16 * h + j, 8 * hp + lp] = v
    M2 = np.zeros((128, 128), np.float32)
    for half in range(2):
        for hp in range(8):
            for lp in range(8):
                for l in range(8):
                    for jp in range(16):
                        v = (1.0 if l == lp else 0.0) - (1.0 if l == lp + 1 else 0.0)
                        if v:
                            M2[64 * half + 8 * hp + lp, 16 * l + jp] = v
    return np.concatenate([BD, W4, M2], axis=1).astype(np.float32)


def _build():
    nc = bacc.Bacc("TRN2", target_bir_lowering=False, debug=False,
                   num_devices=N_CORES)
    img = nc.dram_tensor("img", [SLABS, P, F], F32, kind="ExternalInput")
    params = nc.dram_tensor("params", [N_PARAMS], F32, kind="ExternalInput")
    cdram = nc.dram_tensor("consts", [P, 320], F32, kind="ExternalInput")
    out = nc.dram_tensor("out", [IMG_PER_CORE, OUT], F32, kind="ExternalOutput")

    img_ap = img.ap()
    par_ap = params.ap()
    out_ap = out.ap()

    with tile.TileContext(nc) as tc:
        with (
            tc.tile_pool(name="work", bufs=1) as work,
            tc.tile_pool(name="psum", bufs=1, space="PSUM") as psum,
        ):
            cst = work.tile([P, 320], F32, tag="cst")
            nc.sync.dma_start(out=cst[:], in_=cdram.ap())
            BD = cst[:, 0:128]
            W4 = cst[:, 128:192]
            M2 = cst[:, 192:320]

            # --- persistent plane buffers (manual double-buffer) ---
            A_bufs = [work.tile([P, KPC, 128], BF16, tag=f"A{i}", name=f"A{i}")
                      for i in range(2)]
            B_bufs = [work.tile([P, 8, C], BF16, tag=f"B{i}", name=f"B{i}")
                      for i in range(2)]
            for i in range(2):
                nc.vector.memset(A_bufs[i][:, :, 0:16], 1.0)   # h=0 ones
                nc.vector.memset(B_bufs[i][:, 0, :], 1.0)      # l=0 ones
            x16_bufs = [work.tile([P, F], F16, tag=f"x16_{i}", name=f"x16_{i}")
                        for i in range(2)]
            u32_bufs = [work.tile([P, C], F32, tag=f"u32_{i}", name=f"u32_{i}")
                        for i in range(2)]
            hi_bufs = [work.tile([P, C], F16, tag=f"hi_{i}", name=f"hi_{i}")
                       for i in range(2)]
            y_bufs = [work.tile([P, C], F16, tag=f"y_{i}", name=f"y_{i}")
                      for i in range(2)]

            Pp = [psum.tile([P, 128], F32, tag=f"P{i}", name=f"P{i}") for i in range(2)]
            Hs = psum.tile([P, 128], F32, tag="Hs")
            Pm = work.tile([P, 128], F32, tag="Pm")

            feat_a = work.tile([P, IMG_PER_CORE], F32, tag="feata")
            feat_b = work.tile([BINS, IMG_PER_CORE], F32, tag="featb")

            act_bias = work.tile([P, 8], F32, tag="act_bias")
            for h in range(1, 8):
                nc.gpsimd.memset(
                    act_bias[:, h:h + 1],
                    -(float(h) - 0.5) * float(2.0 ** 38))

            # preload x16 of slab 0
            nc.gpsimd.dma_start(out=x16_bufs[0][:], in_=img_ap[0])

            for s in range(SLABS):
                x16 = x16_bufs[s % 2]
                if s + 1 < SLABS:
                    nc.gpsimd.dma_start(
                        out=x16_bufs[(s + 1) % 2][:], in_=img_ap[s + 1])
                Ps = Pp[s % 2]
                for ck in range(NCHK):
                    par = ck % 2
                    A = A_bufs[par]
                    B = B_bufs[par]
                    u32 = u32_bufs[par]
                    hi = hi_bufs[par]
                    y = y_bufs[par]
                    xc = x16[:, ck * C:(ck + 1) * C]

                    # --- prep ---
                    # u = 8*x + (-0.5 + 2^-10)  (ACT, fp32 out)
                    nc.scalar.activation(
                        out=u32[:], in_=xc, func=ACTF.Copy,
                        scale=8.0, bias=ACT_BIAS_U)
                    # hi = rne(u) via +/- magic (DVE, fp16 out)
                    nc.vector.tensor_scalar(
                        out=hi[:], in0=u32[:], scalar1=MAGIC, scalar2=MAGIC,
                        op0=ALU.add, op1=ALU.subtract)
                    # y = x - hi/8 (DVE stt, fp16)
                    nc.vector.scalar_tensor_tensor(
                        out=y[:], in0=hi[:], scalar=-0.125, in1=xc,
                        op0=ALU.mult, op1=ALU.add)

                    # --- A planes (stationary): [hi >= h - 0.5] ---
                    # layout A[:, k, 16h+j]; write AP A[:, :, 16h:16h+16]
                    for h, eng in ((1, "act"), (2, "act"), (3, "act"),
                                   (4, "gp"), (5, "gp"),
                                   (6, "dve"), (7, "dve")):
                        dst = A[:, :, 16 * h:16 * h + 16]
                        thr = float(h) - 0.5
                        if eng == "act":
                            nc.scalar.activation(
                                out=dst, in_=hi.rearrange("p (k j) -> p k j", j=16)[:],
                                func=ACTF.Sigmoid,
                                scale=float(2.0 ** 38),
                                bias=act_bias[:, h:h + 1])
                        elif eng == "gp":
                            nc.gpsimd.tensor_scalar(
                                out=dst,
                                in0=hi.rearrange("p (k j) -> p k j", j=16)[:],
                                scalar1=thr, scalar2=None, op0=ALU.is_ge)
                        else:
                            nc.vector.tensor_scalar(
                                out=dst,
                                in0=hi.rearrange("p (k j) -> p k j", j=16)[:],
                                scalar1=thr, scalar2=None, op0=ALU.is_ge)

                    # --- B planes (moving): [y >= l/64] ---
                    for l in range(1, 8):
                        nc.vector.tensor_scalar(
                            out=B[:, l, :], in0=y[:],
                            scalar1=float(l) / 64.0, scalar2=None,
                            op0=ALU.is_ge)

                    # --- PE: 64 chained matmuls into PSUM ---
                    for k in range(KPC):
                        nc.tensor.matmul(
                            Ps[:],
                            A[:, k, :],
                            B[:, :, 16 * k:16 * k + 16],
                            start=(ck == 0 and k == 0),
                            stop=(ck == NCHK - 1 and k == KPC - 1),
                        )

                # --- drain slab s: hist at partitions 8h+l ---
                i_img, c_ch = divmod(s, CH)
                nc.vector.tensor_tensor(
                    out=Pm[:], in0=Ps[:], in1=BD, op=ALU.mult)
                half = 1 if c_ch == 1 else 0
                hs_view = Hs[64 * half:64 * half + 64, :]
                nc.tensor.matmul(
                    hs_view, W4, Pm[:], start=True, stop=True)
                if c_ch == 2:
                    dst = feat_b[:, i_img:i_img + 1]
                else:
                    dst = feat_a[64 * c_ch:64 * c_ch + 64, i_img:i_img + 1]
                tmp = work.tile([P, 128], F32, tag="drainT")
                nc.vector.tensor_tensor(
                    out=tmp[64 * half:64 * half + 64, :], in0=hs_view,
                    in1=M2[64 * half:64 * half + 64, :], op=ALU.mult)
                nc.vector.tensor_reduce(
                    out=dst, in_=tmp[64 * half:64 * half + 64, :],
                    axis=mybir.AxisListType.X, op=ALU.add)

            # --- MLP weights from params (as baseline) ---
            w1a = work.tile([P, HID], F32, tag="w1a")
            w1b = work.tile([FEAT - P, HID], F32, tag="w1b")
            nc.sync.dma_start(
                out=w1a[:], in_=par_ap[0:P * HID].rearrange("(a b) -> a b", a=P))
            nc.sync.dma_start(
                out=w1b[:],
                in_=par_ap[P * HID:W1_N].rearrange("(a b) -> a b", a=FEAT - P))
            b1 = work.tile([HID, 1], F32, tag="b1")
            nc.sync.dma_start(
                out=b1[:], in_=par_ap[W1_N:W1_N + B1_N].rearrange(
                    "(a b) -> a b", a=HID))
            w2 = work.tile([HID, OUT], F32, tag="w2")
            nc.sync.dma_start(
                out=w2[:],
                in_=par_ap[W1_N + B1_N:W1_N + B1_N + W2_N].rearrange(
                    "(a b) -> a b", a=HID))
            b2 = work.tile([OUT, 1], F32, tag="b2")
            nc.sync.dma_start(
                out=b2[:],
                in_=par_ap[W1_N + B1_N + W2_N:G_OFF].rearrange(
                    "(a b) -> a b", a=OUT))
            gsc = work.tile([1, 1], F32, tag="gsc")
            nc.sync.dma_start(
                out=gsc[:], in_=par_ap[G_OFF:G_OFF + 1].rearrange(
                    "(a b) -> a b", a=1))
            ones_out = work.tile([1, OUT], F32, tag="ones_out")
            nc.vector.memset(ones_out[:], 1.0)

            g_psum = psum.tile([OUT, 1], F32, tag="gpsum")
            nc.tensor.matmul(g_psum[:], ones_out[:], gsc[:], start=True,
                             stop=True)
            bias2 = work.tile([OUT, 1], F32, tag="bias2")
            nc.vector.tensor_add(out=bias2[:], in0=b2[:], in1=g_psum[:])

            # --- layer 1: h = relu(w1.T @ feat + b1) (transposed) ---
            h_psum = psum.tile([HID, IMG_PER_CORE], F32, tag="hpsum")
            nc.tensor.matmul(h_psum[:], w1a[:], feat_a[:], start=True,
                             stop=False)
            nc.tensor.matmul(h_psum[:], w1b[:], feat_b[:], start=False,
                             stop=True)
            hmlp = work.tile([HID, IMG_PER_CORE], F32, tag="hmlp")
            nc.scalar.activation(
                out=hmlp[:], in_=h_psum[:], func=ACTF.Relu, bias=b1[:],
                scale=1.0)

            # --- layer 2: o = sigmoid(w2.T @ h + b2 + g) ---
            o_psum = psum.tile([OUT, IMG_PER_CORE], F32, tag="opsum")
            nc.tensor.matmul(o_psum[:], w2[:], hmlp[:], start=True, stop=True)
            o = work.tile([OUT, IMG_PER_CORE], F32, tag="o")
            nc.scalar.activation(
                out=o[:], in_=o_psum[:], func=ACTF.Sigmoid, bias=bias2[:],
                scale=1.0)

            nc.sync.dma_start(out=out_ap.rearrange("a b -> b a"), in_=o[:])

    nc.compile()
    return nc


_NC_CACHE = {}


def _get_nc():
    if "nc" not in _NC_CACHE:
        _NC_CACHE["nc"] = _build()
    return _NC_CACHE["nc"]


def make_in_maps(img: np.ndarray, params: np.ndarray):
    shards = img.reshape(N_CORES, SLABS, P, F)
    cst = _consts_np()
    return [
        {"img": shards[c], "params": params, "consts": cst}
        for c in range(N_CORES)
    ]


def kernel(img: np.ndarray, params: np.ndarray) -> np.ndarray:
    img = np.ascontiguousarray(img, dtype=np.float32)
    params = np.ascontiguousarray(params, dtype=np.float32)
    assert img.shape == (N_IMG, CH, 1024, 1024)
    assert params.shape == (N_PARAMS,)

    nc = _get_nc()
    in_maps = make_in_maps(img, params)
    res = bass_utils.run_bass_kernel_spmd(nc, in_maps,
                                          core_ids=list(range(N_CORES)))
    return np.concatenate([res.results[c]["out"] for c in range(N_CORES)],
                          axis=0)


# revision 8
# speedup vs baseline: 2.0943x; 2.0943x over previous
"""Trainium2 Bass kernel for nn_ConditionalModuleBGR (histogram binning + MLP).

Strategy: data-parallel over 8 NeuronCores, 2 images (6 slabs of [128, 8192])
per core. Per slab, the 64-bin histogram is computed as a bilinear form on the
Tensor engine (radix 8x8):

  - x16 = fp16(x); hi = floor(8*x16) via exact magic rounding; y = x16 - hi/8.
  - 7 "coarse" step planes  A_h = [hi >= h]   (plus constant ones plane h=0)
  - 7 "fine" step planes    B_l = [y >= l/64] (plus ones plane l=0)
  - PE accumulates P[16h+j, 16l+j'] += sum_p A_h(e[p,j]) * B_l(e[p,j'])
    over 512 chained matmuls per slab (16 element-columns per matmul).
    Same-element products live on the j==j' diagonal blocks.
  - Tiny drain: block-diag mask, one matmul folding the coarse cumulative
    diff, mask+reduce folding the fine diff -> 64-bin histogram lands at
    partition 8h+l, directly in the MLP feature layout.
  - MLP (192 -> 128 relu -> 32) on PE + ACT as in the reference.

Plane generation is spread across DVE (4x fp16 mode), ACT (sigmoid-step),
and GPSIMD so it overlaps the PE stream and the HBM DMA.
"""

import numpy as np

import concourse.bacc as bacc
import concourse.mybir as mybir
import concourse.tile as tile
from concourse import bass_utils

N_CORES = 8
N_IMG = 16
IMG_PER_CORE = N_IMG // N_CORES  # 2
CH = 3
SLABS = IMG_PER_CORE * CH  # 6
P = 128
F = 8192  # 1024*1024 / 128
BINS = 64
FEAT = CH * BINS  # 192
HID = 128
OUT = 32
W1_N = FEAT * HID
B1_N = HID
W2_N = HID * OUT
B2_N = OUT
G_OFF = W1_N + B1_N + W2_N + B2_N
N_PARAMS = 28864

C = 1024            # columns per chunk
NCHK = F // C       # 8 chunks per slab
KPC = C // 16       # 64 matmuls per chunk

F32 = mybir.dt.float32
F16 = mybir.dt.float16
BF16 = mybir.dt.bfloat16
ALU = mybir.AluOpType
ACTF = mybir.ActivationFunctionType

# floor magic: u = 8x - 0.5 + 2^-10 (exact); hi = rne(u) via +/- 3*2^22
S1_ADD = -(1.0 / 16.0 - 2.0 ** -13)
MAGIC = float(3 * 2 ** 22)
ACT_BIAS_U = -0.5 + 2.0 ** -10  # u = 8*x + bias on ACT


def _consts_np() -> np.ndarray:
    """[128, 320] fp32: BD | W4 | M2 packed along free dim."""
    BD = np.zeros((128, 128), np.float32)
    for h in range(8):
        for l in range(8):
            for j in range(16):
                BD[16 * h + j, 16 * l + j] = 1.0
    W4 = np.zeros((128, 64), np.float32)
    for h in range(8):
        for j in range(16):
            for hp in range(8):
                for lp in range(8):
                    v = (1.0 if h == hp else 0.0) - (1.0 if h == hp + 1 else 0.0)
                    if v:
                        W4[16 * h + j, 8 * hp + lp] = v
    M2 = np.zeros((128, 128), np.float32)
    for half in range(2):
        for hp in range(8):
            for lp in range(8):
                for l in range(8):
                    for jp in range(16):
                        v = (1.0 if l == lp else 0.0) - (1.0 if l == lp + 1 else 0.0)
                        if v:
                            M2[64 * half + 8 * hp + lp, 16 * l + jp] = v
    return np.concatenate([BD, W4, M2], axis=1).astype(np.float32)


def _build():
    nc = bacc.Bacc("TRN2", target_bir_lowering=False, debug=False,
                   num_devices=N_CORES)
    img = nc.dram_tensor("img", [SLABS, P, F], F32, kind="ExternalInput")
    params = nc.dram_tensor("params", [N_PARAMS], F32, kind="ExternalInput")
    cdram = nc.dram_tensor("consts", [P, 320], F32, kind="ExternalInput")
    out = nc.dram_tensor("out", [IMG_PER_CORE, OUT], F32, kind="ExternalOutput")

    img_ap = img.ap()
    par_ap = params.ap()
    out_ap = out.ap()

    with tile.TileContext(nc) as tc:
        with (
            tc.tile_pool(name="work", bufs=1) as work,
            tc.tile_pool(name="psum", bufs=1, space="PSUM") as psum,
        ):
            cst = work.tile([P, 320], F32, tag="cst")
            nc.sync.dma_start(out=cst[:], in_=cdram.ap())
            BD = cst[:, 0:128]
            W4 = cst[:, 128:192]
            M2 = cst[:, 192:320]

            # --- persistent plane buffers (manual double-buffer) ---
            A_bufs = [work.tile([P, KPC, 128], BF16, tag=f"A{i}", name=f"A{i}")
                      for i in range(2)]
            B_bufs = [work.tile([P, 8, C], BF16, tag=f"B{i}", name=f"B{i}")
                      for i in range(2)]
            for i in range(2):
                nc.vector.memset(A_bufs[i][:, :, 0:16], 1.0)   # h=0 ones
                nc.vector.memset(B_bufs[i][:, 0, :], 1.0)      # l=0 ones
            x32_bufs = [work.tile([P, C], F32, tag=f"x32_{i}", name=f"x32_{i}")
                        for i in range(2)]
            x16_bufs = [work.tile([P, C], F16, tag=f"x16_{i}", name=f"x16_{i}")
                        for i in range(2)]
            ast_bufs = [work.tile([P, 3, KPC, 16], BF16, tag=f"ast_{i}",
                                  name=f"ast_{i}")
                        for i in range(2)]
            u32_bufs = [work.tile([P, C], F32, tag=f"u32_{i}", name=f"u32_{i}")
                        for i in range(2)]
            hi_bufs = [work.tile([P, C], F16, tag=f"hi_{i}", name=f"hi_{i}")
                       for i in range(2)]
            y_bufs = [work.tile([P, C], F16, tag=f"y_{i}", name=f"y_{i}")
                      for i in range(2)]

            Pp = [psum.tile([P, 128], F32, tag=f"P{i}", name=f"P{i}") for i in range(2)]
            Hs = psum.tile([P, 128], F32, tag="Hs")
            Pm = work.tile([P, 128], F32, tag="Pm")

            feat_a = work.tile([P, IMG_PER_CORE], F32, tag="feata")
            feat_b = work.tile([BINS, IMG_PER_CORE], F32, tag="featb")

            act_bias = work.tile([P, 8], F32, tag="act_bias")
            for h in range(1, 8):
                nc.gpsimd.memset(
                    act_bias[:, h:h + 1],
                    -(float(h) - 0.5) * float(2.0 ** 38))

            for s in range(SLABS):
                Ps = Pp[s % 2]
                for ck in range(NCHK):
                    par = ck % 2
                    A = A_bufs[par]
                    B = B_bufs[par]
                    Ast = ast_bufs[par]
                    x32 = x32_bufs[par]
                    x16 = x16_bufs[par]
                    u32 = u32_bufs[par]
                    hi = hi_bufs[par]
                    y = y_bufs[par]

                    # --- load fp32 chunk ---
                    nc.sync.dma_start(
                        out=x32[:], in_=img_ap[s, :, ck * C:(ck + 1) * C])

                    # --- prep ---
                    # u = 8*x + (-0.5 + 2^-10)  (ACT, fp32 out)
                    nc.scalar.activation(
                        out=u32[:], in_=x32[:], func=ACTF.Copy,
                        scale=8.0, bias=ACT_BIAS_U)
                    # x16 (DVE copy)
                    nc.vector.tensor_copy(out=x16[:], in_=x32[:])
                    # hi = rne(u) via +/- magic (DVE, fp16 out)
                    nc.vector.tensor_scalar(
                        out=hi[:], in0=u32[:], scalar1=MAGIC, scalar2=MAGIC,
                        op0=ALU.add, op1=ALU.subtract)
                    # y = x - hi/8 (DVE stt, fp16)
                    nc.vector.scalar_tensor_tensor(
                        out=y[:], in0=hi[:], scalar=-0.125, in1=x16[:],
                        op0=ALU.mult, op1=ALU.add)

                    # --- A planes (stationary): [hi >= h - 0.5] ---
                    # h=1..4 on ACT directly into interleaved layout;
                    # h=5..7 dense on DVE then DMA-interleave.
                    hi_v = hi.rearrange("p (k j) -> p k j", j=16)
                    for h in (1, 2, 3, 4):
                        nc.scalar.activation(
                            out=A[:, :, 16 * h:16 * h + 16], in_=hi_v[:],
                            func=ACTF.Sigmoid,
                            scale=float(2.0 ** 38),
                            bias=act_bias[:, h:h + 1])
                    for i, h in enumerate((5, 6, 7)):
                        nc.vector.tensor_scalar(
                            out=Ast[:, i, :, :], in0=hi_v[:],
                            scalar1=float(h) - 0.5, scalar2=None,
                            op0=ALU.is_ge)
                        nc.sync.dma_start(
                            out=A[:, :, 16 * h:16 * h + 16],
                            in_=Ast[:, i, :, :])

                    # --- B planes (moving): [y >= l/64] ---
                    for l in range(1, 8):
                        nc.vector.tensor_scalar(
                            out=B[:, l, :], in0=y[:],
                            scalar1=float(l) / 64.0, scalar2=None,
                            op0=ALU.is_ge)

                    # --- PE: 64 chained matmuls into PSUM ---
                    for k in range(KPC):
                        nc.tensor.matmul(
                            Ps[:],
                            A[:, k, :],
                            B[:, :, 16 * k:16 * k + 16],
                            start=(ck == 0 and k == 0),
                            stop=(ck == NCHK - 1 and k == KPC - 1),
                        )

                # --- drain slab s: hist at partitions 8h+l ---
                i_img, c_ch = divmod(s, CH)
                nc.vector.tensor_tensor(
                    out=Pm[:], in0=Ps[:], in1=BD, op=ALU.mult)
                half = 1 if c_ch == 1 else 0
                hs_view = Hs[64 * half:64 * half + 64, :]
                nc.tensor.matmul(
                    hs_view, W4, Pm[:], start=True, stop=True)
                if c_ch == 2:
                    dst = feat_b[:, i_img:i_img + 1]
                else:
                    dst = feat_a[64 * c_ch:64 * c_ch + 64, i_img:i_img + 1]
                tmp = work.tile([P, 128], F32, tag="drainT")
                nc.vector.tensor_tensor(
                    out=tmp[64 * half:64 * half + 64, :], in0=hs_view,
                    in1=M2[64 * half:64 * half + 64, :], op=ALU.mult)
                nc.vector.tensor_reduce(
                    out=dst, in_=tmp[64 * half:64 * half + 64, :],
                    axis=mybir.AxisListType.X, op=ALU.add)

            # --- MLP weights from params (as baseline) ---
            w1a = work.tile([P, HID], F32, tag="w1a")
            w1b = work.tile([FEAT - P, HID], F32, tag="w1b")
            nc.sync.dma_start(
                out=w1a[:], in_=par_ap[0:P * HID].rearrange("(a b) -> a b", a=P))
            nc.sync.dma_start(
                out=w1b[:],
                in_=par_ap[P * HID:W1_N].rearrange("(a b) -> a b", a=FEAT - P))
            b1 = work.tile([HID, 1], F32, tag="b1")
            nc.sync.dma_start(
                out=b1[:], in_=par_ap[W1_N:W1_N + B1_N].rearrange(
                    "(a b) -> a b", a=HID))
            w2 = work.tile([HID, OUT], F32, tag="w2")
            nc.sync.dma_start(
                out=w2[:],
                in_=par_ap[W1_N + B1_N:W1_N + B1_N + W2_N].rearrange(
                    "(a b) -> a b", a=HID))
            b2 = work.tile([OUT, 1], F32, tag="b2")
            nc.sync.dma_start(
                out=b2[:],
                in_=par_ap[W1_N + B1_N + W2_N:G_OFF].rearrange(
                    "(a b) -> a b", a=OUT))
            gsc = work.tile([1, 1], F32, tag="gsc")
            nc.sync.dma_start(
                out=gsc[:], in_=par_ap[G_OFF:G_OFF + 1].rearrange(
                    "(a b) -> a b", a=1))
            ones_out = work.tile([1, OUT], F32, tag="ones_out")
            nc.vector.memset(ones_out[:], 1.0)

            g_psum = psum.tile([OUT, 1], F32, tag="gpsum")
            nc.tensor.matmul(g_psum[:], ones_out[:], gsc[:], start=True,
                             stop=True)
            bias2 = work.tile([OUT, 1], F32, tag="bias2")
            nc.vector.tensor_add(out=bias2[:], in0=b2[:], in1=g_psum[:])

            # --- layer 1: h = relu(w1.T @ feat + b1) (transposed) ---
            h_psum = psum.tile([HID, IMG_PER_CORE], F32, tag="hpsum")
            nc.tensor.matmul(h_psum[:], w1a[:], feat_a[:], start=True,
                             stop=False)
            nc.tensor.matmul(h_psum[:], w1b[:], feat_b[:], start=False,
                             stop=True)
            hmlp = work.tile([HID, IMG_PER_CORE], F32, tag="hmlp")
            nc.scalar.activation(
                out=hmlp[:], in_=h_psum[:], func=ACTF.Relu, bias=b1[:],
                scale=1.0)

            # --- layer 2: o = sigmoid(w2.T @ h + b2 + g) ---
            o_psum = psum.tile([OUT, IMG_PER_CORE], F32, tag="opsum")
            nc.tensor.matmul(o_psum[:], w2[:], hmlp[:], start=True, stop=True)
            o = work.tile([OUT, IMG_PER_CORE], F32, tag="o")
            nc.scalar.activation(
                out=o[:], in_=o_psum[:], func=ACTF.Sigmoid, bias=bias2[:],
                scale=1.0)

            nc.sync.dma_start(out=out_ap.rearrange("a b -> b a"), in_=o[:])

    nc.compile()
    return nc


_NC_CACHE = {}


def _get_nc():
    if "nc" not in _NC_CACHE:
        _NC_CACHE["nc"] = _build()
    return _NC_CACHE["nc"]


def make_in_maps(img: np.ndarray, params: np.ndarray):
    shards = img.reshape(N_CORES, SLABS, P, F)
    cst = _consts_np()
    return [
        {"img": shards[c], "params": params, "consts": cst}
        for c in range(N_CORES)
    ]


def kernel(img: np.ndarray, params: np.ndarray) -> np.ndarray:
    img = np.ascontiguousarray(img, dtype=np.float32)
    params = np.ascontiguousarray(params, dtype=np.float32)
    assert img.shape == (N_IMG, CH, 1024, 1024)
    assert params.shape == (N_PARAMS,)

    nc = _get_nc()
    in_maps = make_in_maps(img, params)
    res = bass_utils.run_bass_kernel_spmd(nc, in_maps,
                                          core_ids=list(range(N_CORES)))
    return np.concatenate([res.results[c]["out"] for c in range(N_CORES)],
                          axis=0)
